# revision 17
# baseline (speedup 1.0000x reference)
"""Masked-softmax cross-entropy loss on 8 Trainium2 cores — fp8 + 3-engine exp.

Math per target row t (16384 rows, 4096 src cols):
  numer[t] = sum_j exp(x[t,j]/tau) over valid src cols j with color == tgt color t
  denom[t] = sum_j exp(x[t,j]/tau) over valid src cols j
  p_gt = numer/denom, nll = -log(p_gt + eps); rows with numer == 0 masked out.

Device formulation (per core: one batch-half, 2048 target rows):
  x is uploaded TRANSPOSED [4096_j, 2048_t] as fp8e4m3 (halves HBM traffic
  vs bf16; validated end-to-end rel err ~3.5e-3 vs the 2e-2 gate), pre-tiled
  into 8 chunks of [128, 8192] = 4 j-tiles side by side, split per chunk
  into three engine pieces so exp runs on THREE engines concurrently:
    ACT : exact exp LUT, fp8 in -> bf16 out        (~153.6 Gelem/s)
    DVE : Schraudolph exp — tensor_scalar computes u = x*1846.65 + 16256
          and the int16 round-to-nearest writeback IS the bf16 bit pattern
          of 2^(10*x*log2e) (bitcast; validated on HW)   (~123 Gelem/s fp8-in)
    Pool: same Schraudolph trick on gpsimd          (~92 Gelem/s at 0.6 eff)
  PE  : bucket sums S[c,t] = sum_j onehot[j,c] * et[j,t], one-hot shipped
        as fp8 (0/1 exact; mixed fp8-stationary x bf16-moving matmul
        validated exact on HW), accumulated in one [99,2048] f32 PSUM
        region; 4 matmuls per j-tile (PSUM-bank-sized 512-col m-blocks),
        jt-major order so consecutive matmuls share stationary weights.
  Tail: 4 ACT copies PSUM->SBUF (bf16) + 4 sync-ring stores.  Host gathers
  numer[t] = S[tid_t, t], denom[t] = S[98, t], runs the 16K-scalar finalize.

Engine budget per core: DMA 8.4MB on the sync HWDGE ring ~28us (the wall),
ACT ~24us, DVE ~21us, Pool ~23us, PE ~27us at full p-state — all overlapped.

Schedule: engine pieces are 512-aligned (3584/2560/2048) so every matmul's
moving slice lives in exactly one engine's output tile -> every matmul
carries exactly one exp sem wait.  All tiles write-once (fp8 in + bf16 out
both resident; ~200KB/partition).  The one-hot rides the sync ring FIRST
(fp8: 405KB, 1.4us) and a PE warm-up ldweights absorbs its DMA wait.
"""

import os
import numpy as np

B = 4
S_TGT = 8
L_TGT = 512
C = 4
N = 4096          # src columns
P = 128
ROWS = 2048       # tgt rows per core (half a batch)
NCORES = 8
PAD = -1.0
EPS = 1e-15

KC = 99           # one-hot columns: 98 color ids + 1 valid-mask column
NJT = N // P      # 32 j-tiles
JPC = 4           # j-tiles per chunk
NCHUNK = NJT // JPC   # 8 chunks of [128, 8192]
CW = JPC * ROWS   # chunk free width = 8192
MBLK = 512        # matmul moving block (one PSUM bank of f32)

# per-chunk engine split (columns, 512-aligned): ACT / DVE / Pool
SPLIT_A = 3584
SPLIT_D = 2560
SPLIT_P = CW - SPLIT_A - SPLIT_D   # 2048

# Schraudolph constants: u16 = round(x * 10*log2e*128 + 127*128) viewed as bf16
LOG2E = 1.4426950408889634
SCH_SCALE = float(np.float32(10.0 * LOG2E * 128.0))
SCH_BIAS = float(np.float32(127.0 * 128.0))

_NC_CACHE = {}


def _patch_split_drain():
    """Split the kernel-tail drain's sem waits across several drain
    instructions (walrus rejects >1 sync wait on one CTRL instruction)."""
    import concourse.tile as tile
    from concourse.vector_clock import ScopedClock, VectorClock

    if getattr(tile.TileContext, "_split_drain_patched", False):
        return

    def _drain_and_barrier(self, tick_clock, wait_clock):
        g = tick_clock.global_clock
        n = len(g)
        for base in range(n):
            vec = [g[i] if i == base else 0 for i in range(n)]
            if not any(vec):
                continue
            d = self.nc.sync.drain()
            wait_clock.add_sem_waits(d.ins, ScopedClock({None: VectorClock(vec)}))
        self.nc.all_engine_barrier()
        popped = self.nc._tile_sem_poison_stack.pop()
        assert popped is self._sem_poison
        self.nc.clear_and_free_semaphores(list(self.sems.allocated().values()))
        self.nc.all_engine_barrier()

    tile.TileContext._drain_and_barrier = _drain_and_barrier
    tile.TileContext._split_drain_patched = True


def _build_nc():
    import concourse.bass as bass
    import concourse.mybir as mybir
    import concourse.tile as tile
    from contextlib import ExitStack

    _patch_split_drain()
    nc = bass.Bass()
    f32 = mybir.dt.float32
    bf16 = mybir.dt.bfloat16
    f8 = mybir.dt.float8e4
    i16 = mybir.dt.int16

    x = nc.declare_dram_parameter("x", [NCHUNK * P, CW], f8, isOutput=False)
    oneh_d = nc.declare_dram_parameter("oneh", [P, NJT * KC], f8,
                                       isOutput=False)
    s_out = nc.declare_dram_parameter("s", [KC, ROWS], bf16, isOutput=True)

    # Engine column regions per chunk (512-aligned): ACT [0,3584) |
    # DVE [3584,6144) | Pool [6144,8192) — the v1 split that measured
    # balanced under load (ACT 0.92 ns/col stable, DVE ~1.0-1.12, Pool
    # ~1.45).  Head chunks are cut finer so exp overlaps delivery and the
    # PE can start ~12.5us in fully fed (micro-gaps reset the PE p-state
    # ramp, so a slightly later gapless start beats an earlier stalling
    # one); tail chunk finer so the last matmuls chase piece-sized exps.
    def chunk_pieces(ci):
        if ci == 0:
            a = [512, 1024, 2048]
            d = [1024, 1536]
            pp = [1024, 1024]
        elif ci == 1:
            # lighter ACT share here: the ACT queue's head prelude
            # (ring dispatches + table load) makes its c1 exp the last
            # supply to arrive; DVE has slack at this point
            a = [1536, 1024]
            d = [1536, 1024, 1024]
            pp = [1024, 1024]
        elif ci == NCHUNK - 1:
            a = [2048, 1536]
            d = [1536, 1024]
            pp = [1024, 1024]
        else:
            a, d, pp = [3584], [2560], [2048]
        out = []
        c0 = 0
        for w in a:
            out.append(("a", c0, w)); c0 += w
        for w in d:
            out.append(("d", c0, w)); c0 += w
        for w in pp:
            out.append(("p", c0, w)); c0 += w
        assert c0 == CW
        return out

    with tile.TileContext(nc) as tc:
        with ExitStack() as ctx:
            const_pool = ctx.enter_context(tc.tile_pool(name="const", bufs=1))
            x_pool = ctx.enter_context(tc.tile_pool(name="x", bufs=1))
            e_pool = ctx.enter_context(tc.tile_pool(name="e", bufs=1))
            res_pool = ctx.enter_context(tc.tile_pool(name="res", bufs=1))
            psum_pool = ctx.enter_context(
                tc.tile_pool(name="psum", bufs=1, space="PSUM")
            )

            oneh = const_pool.tile([P, NJT * KC], f8)
            # one PSUM tile (= one bank) per m-block: keeps each m-block's
            # copy dependent only on its own last matmul (a shared tile
            # couples the staggered tail copies into >1 sem wait)
            spsum = [psum_pool.tile([P, MBLK], f32, name=f"ps{m}", tag=f"ps{m}")
                     for m in range(4)]
            warm_ps = psum_pool.tile([P, MBLK], f32, name="pswarm", tag="pswarm")
            ssb = [res_pool.tile([P, MBLK], bf16, name=f"sb{m}", tag=f"sb{m}")
                   for m in range(4)]

            pieces = {ci: chunk_pieces(ci) for ci in range(NCHUNK)}
            xt, et = {}, {}
            for ci in range(NCHUNK):
                for tag, col0, w in pieces[ci]:
                    key = (ci, col0)
                    xt[key] = x_pool.tile([P, w], f8, name=f"x{tag}{ci}_{col0}",
                                          tag=f"x{tag}{ci}_{col0}")
                    et[key] = e_pool.tile([P, w], bf16, name=f"e{tag}{ci}_{col0}",
                                          tag=f"e{tag}{ci}_{col0}")

            def load(ci, col0, w, eng):
                eng.dma_start(xt[(ci, col0)][:],
                              x[ci * P:(ci + 1) * P, col0:col0 + w])

            # ---- two DMA rings.  Sync HWDGE: first 2 j-tiles of one-hot
            # (25KB — gates the first matmul), then x pieces in consumption
            # order.  ACT HWDGE: chunk 0/1 DVE pieces (banks extra supply
            # at the head, in parallel with the sync stream) + one-hot
            # tail; its dispatches sit ahead of the ACT table-load + exps,
            # which the PE does not need until ~12.5us.
            OH1 = 2 * KC
            nc.sync.dma_start(oneh[:, 0:OH1], oneh_d[:, 0:OH1])
            for ci in range(NCHUNK):
                for tag, col0, w in pieces[ci]:
                    if tag == "d" and ci == 0:
                        load(ci, col0, w, nc.scalar)
                    else:
                        load(ci, col0, w, nc.sync)
            nc.scalar.dma_start(oneh[:, OH1:], oneh_d[:, OH1:])
            # PE warm-up ldweights absorbs the one-hot head piece's DMA
            # wait; ~22 throwaway matmuls (one-hot head as both operands,
            # scratch PSUM bank) delay the real stream to ~13.6us — where
            # the exp supply curve stays ahead for good — while carrying
            # the PE up its p-state ramp.
            nc.tensor.ldweights(oneh[:, 0:KC])
            for _ in range(22):
                nc.tensor.matmul(
                    warm_ps[0:KC, 0:OH1], oneh[:, 0:KC], oneh[:, 0:OH1],
                    start=True, stop=True)

            def moving_slice(ci, g0):
                for tag, col0, w in pieces[ci]:
                    if col0 <= g0 < col0 + w:
                        assert g0 + MBLK <= col0 + w
                        return et[(ci, col0)][:, g0 - col0:g0 - col0 + MBLK]
                raise AssertionError(g0)

            def exp_piece(ci, col0):
                tag = next(t for t, c0, w in pieces[ci] if c0 == col0)
                xin, eout = xt[(ci, col0)], et[(ci, col0)]
                if tag == "a":
                    nc.scalar.activation(
                        eout[:], xin[:],
                        mybir.ActivationFunctionType.Exp, scale=10.0)
                elif tag == "d":
                    nc.vector.tensor_scalar(
                        eout[:].bitcast(i16), xin[:], SCH_SCALE, SCH_BIAS,
                        mybir.AluOpType.mult, mybir.AluOpType.add)
                else:
                    nc.gpsimd.tensor_scalar(
                        eout[:].bitcast(i16), xin[:], SCH_SCALE, SCH_BIAS,
                        mybir.AluOpType.mult, mybir.AluOpType.add)

            nmm = NJT
            seen = {0: 0, 1: 0, 2: 0, 3: 0}

            for ci in range(NCHUNK):
                for tag, c0, w in pieces[ci]:
                    exp_piece(ci, c0)
                if ci == 0:
                    # one-hot tail (jt2+) warm-up: carries that DMA sem
                    # wait once so later matmuls stay at one wait each
                    nc.tensor.ldweights(oneh[:, 2 * KC:2 * KC + KC])

                if ci < NCHUNK - 1:
                    order = [(l, mb) for l in range(JPC) for mb in range(4)]
                else:
                    order = [(l, mb) for mb in range(4) for l in range(JPC)]
                for l, mb in order:
                    jt = ci * JPC + l
                    g0 = l * ROWS + mb * MBLK
                    seen[mb] += 1
                    nc.tensor.matmul(
                        spsum[mb][0:KC, :],
                        oneh[:, jt * KC:(jt + 1) * KC],
                        moving_slice(ci, g0),
                        start=(seen[mb] == 1),
                        stop=(seen[mb] == nmm),
                    )
                    if seen[mb] == nmm:
                        # copy on DVE (tail slack; ACT is maxed) and store
                        # via SWDGE: each carries exactly one sem wait
                        nc.vector.tensor_copy(
                            ssb[mb][0:KC, :],
                            spsum[mb][0:KC, :])
                        nc.gpsimd.dma_start(
                            s_out[:, mb * MBLK:(mb + 1) * MBLK],
                            ssb[mb][0:KC, :])
    return nc


def _get_nc():
    if "nc" not in _NC_CACHE:
        _NC_CACHE["nc"] = _build_nc()
    return _NC_CACHE["nc"]


def _color_ids(src, tgt):
    """Map each color row to a per-batch integer id via exact byte equality."""
    src_f = np.ascontiguousarray(src.reshape(B, -1, C))
    tgt_f = np.ascontiguousarray(tgt.reshape(B, -1, C))
    n_s = src_f.shape[1]
    src_ids = np.empty((B, n_s), np.int64)
    tgt_ids = np.empty((B, tgt_f.shape[1]), np.int64)
    for b in range(B):
        allc = np.ascontiguousarray(np.concatenate([src_f[b], tgt_f[b]], axis=0))
        view = allc.view([("", allc.dtype)] * C).reshape(-1)
        _, inv = np.unique(view, return_inverse=True)
        s_ids, t_ids = inv[:n_s].copy(), inv[n_s:].copy()
        s_ids[np.all(src_f[b] == PAD, axis=-1)] = -1
        t_ids[np.all(tgt_f[b] == PAD, axis=-1)] = -2
        src_ids[b], tgt_ids[b] = s_ids, t_ids
    return src_ids, tgt_ids


def kernel(seg_sim_map, seg_colors_src, seg_colors_tgt):
    import ml_dtypes
    from concourse.bass_utils import run_bass_kernel_spmd

    f8 = ml_dtypes.float8_e4m3
    seg_sim_map = np.asarray(seg_sim_map, dtype=np.float32)
    src_ids, tgt_ids = _color_ids(
        np.asarray(seg_colors_src, np.float32), np.asarray(seg_colors_tgt, np.float32)
    )
    assert src_ids.max() < KC - 1, "color id overflows one-hot width"

    # per-batch one-hot color matrix [N, KC]: col c<98 = (sid == c),
    # col 98 = valid mask; pad columns are all-zero -> excluded exactly.
    oneh_b = []
    for b in range(B):
        oh = np.zeros((N, KC), np.float32)
        valid = src_ids[b] >= 0
        oh[np.arange(N)[valid], src_ids[b][valid]] = 1.0
        oh[valid, KC - 1] = 1.0
        oneh_b.append(
            np.ascontiguousarray(
                oh.reshape(NJT, P, KC).transpose(1, 0, 2).reshape(P, NJT * KC)
            ).astype(f8)
        )

    in_maps = []
    for c in range(NCORES):
        b, h = c // 2, c % 2
        rows = slice(h * ROWS, (h + 1) * ROWS)
        xT = seg_sim_map[b, rows, :].T.astype(f8)              # [N, ROWS]
        xh = np.ascontiguousarray(
            xT.reshape(NCHUNK, JPC, P, ROWS)
            .transpose(0, 2, 1, 3)
            .reshape(NCHUNK * P, CW)
        )
        in_maps.append({"x": xh, "oneh": oneh_b[b]})

    trace = os.environ.get("KERNEL_PROFILE", "") == "1"
    nc = _get_nc()
    out = run_bass_kernel_spmd(nc, in_maps, list(range(NCORES)), trace=trace)
    if trace and out.exec_time_ns is not None:
        print(f"HW exec time: {out.exec_time_ns} ns")
        print(f"HW exec mean: {out.mean_exec_time_ns} ns")

    numer = np.empty((B, N), np.float32)
    denom = np.empty((B, N), np.float32)
    for c in range(NCORES):
        b, h = c // 2, c % 2
        rows = slice(h * ROWS, (h + 1) * ROWS)
        r = out.results[c]
        S = np.asarray(r["s"], np.float32)
        tid = tgt_ids[b, rows]
        valid_t = tid >= 0
        nm = np.zeros(ROWS, np.float32)
        nm[valid_t] = S[tid[valid_t], np.arange(ROWS)[valid_t]]
        numer[b, rows] = nm
        denom[b, rows] = S[KC - 1, :]

    # host finalize, mirroring the reference ops in f32 (touches 16K scalars)
    p_gt = numer / denom
    nll = -np.log(p_gt + np.float32(EPS))
    m = (numer > 0).astype(np.float32)
    nll3 = nll.reshape(B, S_TGT, L_TGT)
    m3 = m.reshape(B, S_TGT, L_TGT)
    nvalid = m3.sum(-1)
    seg_loss = np.where(
        nvalid > 0, (nll3 * m3).sum(-1) / np.maximum(nvalid, np.float32(1.0)), 0.0
    ).astype(np.float32)
    cnt = int((nvalid > 0).sum())
    total = np.float32(seg_loss.sum(dtype=np.float32) / np.float32(max(cnt, 1)))
    return np.asarray(total, np.float32), np.asarray(cnt, np.int32)
